# Initial kernel scaffold
#
"""AxialAttention TRN2 Bass kernel — 8-core data-parallel over batch.

Reference math (per batch element b, per head h):
  qkv = x @ w_qkv;  q,k,v split; heads of dh=64
  S[m, n] = q_m . k_n / 8   (m, n over 1024 = 32x32 positions)
  attn = softmax over y only, where n = x*32 + y  (groups of 32 consecutive n)
  out[m] = sum_n attn[m, n] v[n];  y = out @ w_out + b_out

Per-core layout strategy (core c handles batch c):
  xT   [512, 1024]  (PE transpose of x)      f32r
  qkT  [1024, 1024] = (x @ w_qkv[:, :1024]).T (q rows 0-511, k rows 512-1023)
  v    [1024, 512]  natural layout
  E^T  per (head, n-tile): [128 n, 1024 m] = exp(S^T/8)  (ACT, from PSUM)
  Z    [32 g, 1024 m] per head: group sums via accumulated indicator matmuls
  rz   = 1/Z (gpsimd divide), relocated to 32-aligned strips (SBUF->SBUF DMA)
  R    [128, 1024] = rz broadcast across partition groups (row-packed matmuls)
  E'   = E * R (DVE, in place)
  outT per head [64 d, 1024 m] = v_h.T-contract: lhsT=v, rhs=E'
  y    = outT.T @ w_out + b_out (bias via K=1 matmul), PSUM->SBUF->DRAM
"""
import numpy as np

B, H, W, DIM = 8, 32, 32, 512
HEADS, DH = 8, 64
M = H * W          # 1024 query/key positions
NT = M // 128      # 8 n-tiles / m-tiles / pos-tiles

_CACHE = {}


def _build(loop_n=1, parts="all"):
    import concourse.bass as bass
    import concourse.mybir as mybir
    import concourse.tile as tile
    from concourse import bacc
    from contextlib import ExitStack

    F32 = mybir.dt.float32
    F32R = mybir.dt.float32r
    EXP = mybir.ActivationFunctionType.Exp
    COPY = mybir.ActivationFunctionType.Copy

    nc = bacc.Bacc("TRN2", target_bir_lowering=False, debug=False,
                   enable_asserts=False, num_devices=8)
    x = nc.dram_tensor("x", [M, DIM], F32, kind="ExternalInput").ap()
    w_qkv = nc.dram_tensor("w_qkv", [DIM, 3 * DIM], F32R, kind="ExternalInput").ap()
    w_out = nc.dram_tensor("w_out", [DIM, DIM], F32R, kind="ExternalInput").ap()
    b_out = nc.dram_tensor("b_out", [1, DIM], F32R, kind="ExternalInput").ap()
    ident = nc.dram_tensor("ident", [128, 128], F32, kind="ExternalInput").ap()
    i4all = nc.dram_tensor("i4all", [128, 256], F32R, kind="ExternalInput").ap()
    sel = nc.dram_tensor("sel", [128, 128], F32R, kind="ExternalInput").ap()
    ones_r = nc.dram_tensor("ones_r", [1, M], F32R, kind="ExternalInput").ap()
    y = nc.dram_tensor("y", [M, DIM], F32, kind="ExternalOutput").ap()

    with tile.TileContext(nc) as tc, ExitStack() as top:
        if loop_n > 1:
            top.enter_context(tc.For_i(0, loop_n, 1))
        persist = top.enter_context(tc.tile_pool(name="persist", bufs=1))

        # ---- persistent constants ----
        ident_sb = persist.tile([128, 128], F32, tag="ident")
        nc.sync.dma_start(out=ident_sb, in_=ident)
        i4_sb = persist.tile([128, 256], F32R, tag="i4")
        nc.sync.dma_start(out=i4_sb, in_=i4all)
        sel_sb = persist.tile([128, 128], F32R, tag="sel")
        nc.sync.dma_start(out=sel_sb, in_=sel)
        ones_sb = persist.tile([1, M], F32R, tag="ones")
        nc.sync.dma_start(out=ones_sb, in_=ones_r)
        bout_sb = persist.tile([1, DIM], F32R, tag="bout")
        nc.sync.dma_start(out=bout_sb, in_=b_out)
        wo_sb = [persist.tile([128, DIM], F32R, tag=f"wo{i}", name=f"wo{i}") for i in range(4)]
        for i in range(4):
            nc.sync.dma_start(out=wo_sb[i], in_=w_out[128 * i:128 * (i + 1), :])

        # ---- persistent activations ----
        qkT_sb = [persist.tile([128, M], F32R, tag=f"qkT{i}", name=f"qkT{i}") for i in range(8)]
        v_sb = [persist.tile([128, DIM], F32R, tag=f"v{i}", name=f"v{i}") for i in range(NT)]
        outT_sb = [persist.tile([128, M], F32R, tag=f"outT{i}", name=f"outT{i}") for i in range(4)]

        # ================= FRONT =================
        with ExitStack() as fr:
            fsb = fr.enter_context(tc.tile_pool(name="front_sb", bufs=1))
            xt_pool = fr.enter_context(tc.tile_pool(name="xt_ps", bufs=2, space="PSUM"))
            mm_pool = fr.enter_context(tc.tile_pool(name="mm_ps", bufs=2, space="PSUM"))

            x_sb = [fsb.tile([128, DIM], F32, tag=f"x{mt}", name=f"x{mt}") for mt in range(NT)]
            for mt in range(NT):
                nc.sync.dma_start(out=x_sb[mt], in_=x[128 * mt:128 * (mt + 1), :])
            wq_sb = [fsb.tile([128, 3 * DIM], F32R, tag=f"wq{kt}", name=f"wq{kt}") for kt in range(4)]
            for kt in range(4):
                nc.sync.dma_start(out=wq_sb[kt], in_=w_qkv[128 * kt:128 * (kt + 1), :])

            # x transpose: xT[kc] [128 k, 1024 m]
            xT_sb = []
            for kc in range(4):
                xt_ps = xt_pool.tile([128, M], F32, tag="xt")
                for mt in range(NT):
                    nc.tensor.matmul(xt_ps[:, 128 * mt:128 * (mt + 1)],
                                     x_sb[mt][:, 128 * kc:128 * (kc + 1)],
                                     ident_sb, is_transpose=True,
                                     start=True, stop=True)
                t = fsb.tile([128, M], F32R, tag=f"xT{kc}", name=f"xT{kc}")
                nc.vector.tensor_copy(t, xt_ps)
                xT_sb.append(t)

            # qkT[ft] = (x @ w_qkv[:, :1024]).T f-tile ft
            for ft in range(8):
                qk_ps = mm_pool.tile([128, M], F32, tag="mm")
                for mc in range(2):
                    for kt in range(4):
                        nc.tensor.matmul(
                            qk_ps[:, 512 * mc:512 * (mc + 1)],
                            wq_sb[kt][:, 128 * ft:128 * (ft + 1)],
                            xT_sb[kt][:, 512 * mc:512 * (mc + 1)],
                            start=(kt == 0), stop=(kt == 3))
                nc.vector.tensor_copy(qkT_sb[ft], qk_ps)

            # v natural: v[pt] [128 pos, 512 vf]
            for pt in range(NT):
                v_ps = mm_pool.tile([128, DIM], F32, tag="mm")
                for kt in range(4):
                    nc.tensor.matmul(v_ps,
                                     xT_sb[kt][:, 128 * pt:128 * (pt + 1)],
                                     wq_sb[kt][:, 1024:1536],
                                     start=(kt == 0), stop=(kt == 3))
                nc.vector.tensor_copy(v_sb[pt], v_ps)

        # ================= ATTENTION =================
        if parts != "front":
          with ExitStack() as at:
            e_pool = at.enter_context(tc.tile_pool(name="e_sb", bufs=21))
            stg_pool = at.enter_context(tc.tile_pool(name="stg_sb", bufs=2))
            rz_pool = at.enter_context(tc.tile_pool(name="rz_sb", bufs=2))
            rzs_pool = at.enter_context(tc.tile_pool(name="rzs_sb", bufs=4))
            s_pool = at.enter_context(tc.tile_pool(name="s_ps", bufs=2, space="PSUM"))
            r_pool = at.enter_context(tc.tile_pool(name="r_ps", bufs=2, space="PSUM"))
            z_pool = at.enter_context(tc.tile_pool(name="z_ps", bufs=1, space="PSUM"))
            pv_pool = at.enter_context(tc.tile_pool(name="pv_ps", bufs=1, space="PSUM"))

            E = {}       # (s, hh, nt) -> tile
            stages = {}

            def alpha_chunks(s):
                out = []
                for nt in range(NT):
                    for hh in range(2):
                        def emit(s=s, nt=nt, hh=hh):
                            h = 2 * s + hh
                            off = 64 * (h % 2)
                            qt = qkT_sb[h // 2]
                            kt_ = qkT_sb[4 + h // 2]
                            s_ps = s_pool.tile([128, M], F32, tag="s",
                                               name=f"sps_{s}_{nt}_{hh}")
                            for mc in range(2):
                                nc.tensor.matmul(
                                    s_ps[:, 512 * mc:512 * (mc + 1)],
                                    kt_[off:off + 64, 128 * nt:128 * (nt + 1)],
                                    qt[off:off + 64, 512 * mc:512 * (mc + 1)],
                                    start=True, stop=True,
                                    tile_position=(off, 0))
                            e = e_pool.tile([128, M], F32R, tag="E",
                                            name=f"E_{s}_{nt}_{hh}")
                            nc.scalar.activation(out=e, in_=s_ps, func=EXP,
                                                 scale=0.125)
                            E[s, hh, nt] = e
                        out.append(emit)
                return out

            def beta_chunks(s):
                chunks = []

                def mkstage(s=s):
                    stages[s] = stg_pool.tile([64, M], F32R, tag="stg",
                                              name=f"stg{s}")
                chunks.append(mkstage)
                for hh in range(2):
                    for mc in range(2):
                        unit = {}

                        def zblock(s=s, hh=hh, mc=mc, unit=unit):
                            mcs = slice(512 * mc, 512 * (mc + 1))
                            z_ps = z_pool.tile([32, 512], F32, tag="z",
                                               name=f"z_{s}_{hh}_{mc}")
                            for nt in range(NT):
                                nc.tensor.matmul(
                                    z_ps, i4_sb[:, 32 * nt:32 * (nt + 1)],
                                    E[s, hh, nt][:, mcs],
                                    start=(nt == 0), stop=(nt == 7))
                            rz32 = rz_pool.tile([32, 512], F32, tag="rz32",
                                                name=f"rz32_{s}_{hh}_{mc}")
                            nc.vector.reciprocal_approx_fast(out=rz32, in_=z_ps)
                            rz = rz_pool.tile([32, 512], F32R, tag="rz",
                                              name=f"rz_{s}_{hh}_{mc}")
                            nc.vector.tensor_copy(rz, rz32)
                            rzs = []
                            for r in range(2):
                                t = rzs_pool.tile([128, 512], F32R, tag="rzs",
                                                  name=f"rzs_{s}_{hh}_{mc}_{r}")
                                for a in range(4):
                                    nc.sync.dma_start(
                                        out=t[32 * a:32 * a + 4, :],
                                        in_=rz[16 * r + 4 * a:
                                               16 * r + 4 * a + 4, :])
                                rzs.append(t)
                            unit["rzs"] = rzs
                            unit["pv"] = pv_pool.tile(
                                [64, 512], F32, tag="pv",
                                name=f"pv_{s}_{hh}_{mc}")
                        chunks.append(zblock)

                        for nt in range(NT):
                            def step(s=s, hh=hh, mc=mc, nt=nt, unit=unit):
                                h = 2 * s + hh
                                mcs = slice(512 * mc, 512 * (mc + 1))
                                r, a = nt // 4, nt % 4
                                r_ps = r_pool.tile([128, 512], F32, tag="r",
                                                   name=f"rps_{s}_{hh}_{mc}_{nt}")
                                nc.tensor.matmul(
                                    r_ps, sel_sb[32 * a:32 * a + 4, :],
                                    unit["rzs"][r][32 * a:32 * a + 4, :],
                                    start=True, stop=True,
                                    tile_position=(32 * a, 0))
                                nc.vector.tensor_mul(
                                    out=E[s, hh, nt][:, mcs],
                                    in0=E[s, hh, nt][:, mcs], in1=r_ps)
                                nc.tensor.matmul(
                                    unit["pv"], v_sb[nt][:, 64 * h:64 * (h + 1)],
                                    E[s, hh, nt][:, mcs],
                                    start=(nt == 0), stop=(nt == 7))
                            chunks.append(step)

                        def copyout(s=s, hh=hh, mc=mc, unit=unit):
                            mcs = slice(512 * mc, 512 * (mc + 1))
                            if hh == 0:
                                nc.scalar.activation(
                                    out=outT_sb[s][0:64, mcs],
                                    in_=unit["pv"], func=COPY)
                            else:
                                nc.scalar.activation(
                                    out=stages[s][:, mcs],
                                    in_=unit["pv"], func=COPY)
                        chunks.append(copyout)

                def merge(s=s):
                    nc.sync.dma_start(out=outT_sb[s][64:128, :], in_=stages[s])
                chunks.append(merge)
                return chunks

            # software-pipeline: alpha(s) interleaved with beta(s-1)
            npair = 4
            for s in range(npair + 1):
                a = alpha_chunks(s) if s < npair and parts != "front" else []
                b = beta_chunks(s - 1) if s >= 1 and parts != "alpha" else []
                na, nb = len(a), len(b)
                if not a:
                    for f in b:
                        f()
                else:
                    ratio = nb / na if na else 0
                    bi = 0.0
                    for i, f in enumerate(a):
                        f()
                        target = (i + 1) * ratio
                        while bi < target and int(bi) < nb:
                            b[int(bi)]()
                            bi += 1
                    for j in range(int(bi), nb):
                        b[j]()

        # ================= PROJ =================
        if parts == "all":
          with ExitStack() as pj:
            pj_pool = pj.enter_context(tc.tile_pool(name="pj_ps", bufs=2, space="PSUM"))
            y_pool = pj.enter_context(tc.tile_pool(name="y_sb", bufs=3))
            for mt in range(NT):
                p = pj_pool.tile([128, DIM], F32, tag="pj")
                for kt in range(4):
                    nc.tensor.matmul(p,
                                     outT_sb[kt][:, 128 * mt:128 * (mt + 1)],
                                     wo_sb[kt], start=(kt == 0), stop=False)
                nc.tensor.matmul(p, ones_sb[:, 128 * mt:128 * (mt + 1)],
                                 bout_sb, start=False, stop=True)
                y_sb = y_pool.tile([128, DIM], F32, tag="y")
                nc.scalar.activation(out=y_sb, in_=p, func=COPY)
                nc.sync.dma_start(out=y[128 * mt:128 * (mt + 1), :], in_=y_sb)

    nc.compile()
    return nc


def _consts():
    ident = np.eye(128, dtype=np.float32)
    i4all = np.zeros((128, 256), np.float32)
    for nt in range(8):
        for p in range(128):
            i4all[p, 32 * nt + 4 * nt + p // 32] = 1.0
    sel = np.zeros((128, 128), np.float32)
    for p in range(128):
        if p % 32 < 4:
            for c in range(128):
                if c // 32 == p % 32:
                    sel[p, c] = 1.0
    ones_r = np.ones((1, M), np.float32)
    return ident, i4all, sel, ones_r


def kernel(x, w_qkv, w_out, b_out):
    from concourse import bass_utils
    if "nc" not in _CACHE:
        _CACHE["nc"] = _build()
    nc = _CACHE["nc"]
    ident, i4all, sel, ones_r = _consts()
    x = np.asarray(x, dtype=np.float32)
    in_maps = []
    for c in range(8):
        in_maps.append({
            "x": np.ascontiguousarray(x[c].reshape(M, DIM)),
            "w_qkv": np.asarray(w_qkv, np.float32),
            "w_out": np.asarray(w_out, np.float32),
            "b_out": np.asarray(b_out, np.float32).reshape(1, DIM),
            "ident": ident, "i4all": i4all, "sel": sel, "ones_r": ones_r,
        })
    res = bass_utils.run_bass_kernel_spmd(nc, in_maps, core_ids=list(range(8)))
    out = np.stack([res.results[c]["y"].reshape(H, W, DIM) for c in range(8)])
    return out



# revision 1
# speedup vs baseline: 4.3189x; 4.3189x over previous
"""AxialAttention TRN2 Bass kernel — 8-core data-parallel over batch.

Reference math (per batch element b, per head h):
  qkv = x @ w_qkv;  q,k,v split; heads of dh=64
  S[m, n] = q_m . k_n / 8   (m, n over 1024 = 32x32 positions)
  attn = softmax over y only, where n = x*32 + y  (groups of 32 consecutive n)
  out[m] = sum_n attn[m, n] v[n];  y = out @ w_out + b_out

Per-core layout strategy (core c handles batch c):
  xT   [512, 1024]  (PE transpose of x)      f32r
  qkT  [1024, 1024] = (x @ w_qkv[:, :1024]).T (q rows 0-511, k rows 512-1023)
  v    [1024, 512]  natural layout
  E^T  per (head, n-tile): [128 n, 1024 m] = exp(S^T/8)  (ACT, from PSUM)
  Z    [32 g, 1024 m] per head: group sums via accumulated indicator matmuls
  rz   = 1/Z (gpsimd divide), relocated to 32-aligned strips (SBUF->SBUF DMA)
  R    [128, 1024] = rz broadcast across partition groups (row-packed matmuls)
  E'   = E * R (DVE, in place)
  outT per head [64 d, 1024 m] = v_h.T-contract: lhsT=v, rhs=E'
  y    = outT.T @ w_out + b_out (bias via K=1 matmul), PSUM->SBUF->DRAM
"""
import numpy as np

B, H, W, DIM = 8, 32, 32, 512
HEADS, DH = 8, 64
M = H * W          # 1024 query/key positions
NT = M // 128      # 8 n-tiles / m-tiles / pos-tiles

_CACHE = {}


def _build(loop_n=1, parts="all"):
    import concourse.bass as bass
    import concourse.mybir as mybir
    import concourse.tile as tile
    from concourse import bacc
    from contextlib import ExitStack

    F32 = mybir.dt.float32
    F32R = mybir.dt.float32r
    EXP = mybir.ActivationFunctionType.Exp
    COPY = mybir.ActivationFunctionType.Copy

    nc = bacc.Bacc("TRN2", target_bir_lowering=False, debug=False,
                   enable_asserts=False, num_devices=8)
    x = nc.dram_tensor("x", [M, DIM], F32, kind="ExternalInput").ap()
    w_qkv = nc.dram_tensor("w_qkv", [DIM, 3 * DIM], F32R, kind="ExternalInput").ap()
    w_out = nc.dram_tensor("w_out", [DIM, DIM], F32R, kind="ExternalInput").ap()
    b_out = nc.dram_tensor("b_out", [1, DIM], F32R, kind="ExternalInput").ap()
    ident = nc.dram_tensor("ident", [128, 128], F32, kind="ExternalInput").ap()
    i4all = nc.dram_tensor("i4all", [128, 256], F32R, kind="ExternalInput").ap()
    sel = nc.dram_tensor("sel", [128, 128], F32R, kind="ExternalInput").ap()
    ones_r = nc.dram_tensor("ones_r", [1, M], F32R, kind="ExternalInput").ap()
    y = nc.dram_tensor("y", [M, DIM], F32, kind="ExternalOutput").ap()

    with tile.TileContext(nc) as tc, ExitStack() as top:
        if loop_n > 1:
            top.enter_context(tc.For_i(0, loop_n, 1))
        persist = top.enter_context(tc.tile_pool(name="persist", bufs=1))

        # ---- persistent constants ----
        ident_sb = persist.tile([128, 128], F32, tag="ident")
        nc.sync.dma_start(out=ident_sb, in_=ident)
        i4_sb = persist.tile([128, 256], F32R, tag="i4")
        nc.sync.dma_start(out=i4_sb, in_=i4all)
        sel_sb = persist.tile([128, 128], F32R, tag="sel")
        nc.sync.dma_start(out=sel_sb, in_=sel)
        ones_sb = persist.tile([1, M], F32R, tag="ones")
        nc.sync.dma_start(out=ones_sb, in_=ones_r)
        bout_sb = persist.tile([1, DIM], F32R, tag="bout")
        nc.sync.dma_start(out=bout_sb, in_=b_out)
        wo_sb = [persist.tile([128, DIM], F32R, tag=f"wo{i}", name=f"wo{i}") for i in range(4)]
        for i in range(4):
            nc.sync.dma_start(out=wo_sb[i], in_=w_out[128 * i:128 * (i + 1), :])

        # ---- persistent activations ----
        qkT_sb = [persist.tile([128, M], F32R, tag=f"qkT{i}", name=f"qkT{i}") for i in range(8)]
        v_sb = [persist.tile([128, DIM], F32R, tag=f"v{i}", name=f"v{i}") for i in range(NT)]
        outT_sb = [persist.tile([128, M], F32R, tag=f"outT{i}", name=f"outT{i}") for i in range(4)]

        # ================= FRONT =================
        with ExitStack() as fr:
            fsb = fr.enter_context(tc.tile_pool(name="front_sb", bufs=1))
            xt_pool = fr.enter_context(tc.tile_pool(name="xt_ps", bufs=2, space="PSUM"))
            mm_pool = fr.enter_context(tc.tile_pool(name="mm_ps", bufs=2, space="PSUM"))

            x_sb = [fsb.tile([128, DIM], F32, tag=f"x{mt}", name=f"x{mt}") for mt in range(NT)]
            for mt in range(NT):
                nc.sync.dma_start(out=x_sb[mt], in_=x[128 * mt:128 * (mt + 1), :])
            wq_sb = [fsb.tile([128, 3 * DIM], F32R, tag=f"wq{kt}", name=f"wq{kt}") for kt in range(4)]
            for kt in range(4):
                nc.sync.dma_start(out=wq_sb[kt], in_=w_qkv[128 * kt:128 * (kt + 1), :])

            # x transpose: xT[kc] [128 k, 1024 m]
            xT_sb = []
            for kc in range(4):
                xt_ps = xt_pool.tile([128, M], F32, tag="xt")
                for mt in range(NT):
                    nc.tensor.matmul(xt_ps[:, 128 * mt:128 * (mt + 1)],
                                     x_sb[mt][:, 128 * kc:128 * (kc + 1)],
                                     ident_sb, is_transpose=True,
                                     start=True, stop=True)
                t = fsb.tile([128, M], F32R, tag=f"xT{kc}", name=f"xT{kc}")
                nc.vector.tensor_copy(t, xt_ps)
                xT_sb.append(t)

            # qkT[ft] = (x @ w_qkv[:, :1024]).T f-tile ft
            for ft in range(8):
                qk_ps = mm_pool.tile([128, M], F32, tag="mm")
                for mc in range(2):
                    for kt in range(4):
                        nc.tensor.matmul(
                            qk_ps[:, 512 * mc:512 * (mc + 1)],
                            wq_sb[kt][:, 128 * ft:128 * (ft + 1)],
                            xT_sb[kt][:, 512 * mc:512 * (mc + 1)],
                            start=(kt == 0), stop=(kt == 3))
                nc.vector.tensor_copy(qkT_sb[ft], qk_ps)

            # v natural: v[pt] [128 pos, 512 vf]
            for pt in range(NT):
                v_ps = mm_pool.tile([128, DIM], F32, tag="mm")
                for kt in range(4):
                    nc.tensor.matmul(v_ps,
                                     xT_sb[kt][:, 128 * pt:128 * (pt + 1)],
                                     wq_sb[kt][:, 1024:1536],
                                     start=(kt == 0), stop=(kt == 3))
                nc.vector.tensor_copy(v_sb[pt], v_ps)

        # ================= ATTENTION =================
        if parts != "front":
          with ExitStack() as at:
            e_pool = at.enter_context(tc.tile_pool(name="e_sb", bufs=21))
            stg_pool = at.enter_context(tc.tile_pool(name="stg_sb", bufs=2))
            rz_pool = at.enter_context(tc.tile_pool(name="rz_sb", bufs=2))
            rzs_pool = at.enter_context(tc.tile_pool(name="rzs_sb", bufs=4))
            s_pool = at.enter_context(tc.tile_pool(name="s_ps", bufs=2, space="PSUM"))
            r_pool = at.enter_context(tc.tile_pool(name="r_ps", bufs=2, space="PSUM"))
            z_pool = at.enter_context(tc.tile_pool(name="z_ps", bufs=1, space="PSUM"))
            pv_pool = at.enter_context(tc.tile_pool(name="pv_ps", bufs=1, space="PSUM"))

            E = {}       # (s, hh, nt) -> tile
            stages = {}

            def alpha_chunks(s):
                out = []
                for nt in range(NT):
                    for hh in range(2):
                        def emit(s=s, nt=nt, hh=hh):
                            h = 2 * s + hh
                            off = 64 * (h % 2)
                            qt = qkT_sb[h // 2]
                            kt_ = qkT_sb[4 + h // 2]
                            s_ps = s_pool.tile([128, M], F32, tag="s",
                                               name=f"sps_{s}_{nt}_{hh}")
                            for mc in range(2):
                                nc.tensor.matmul(
                                    s_ps[:, 512 * mc:512 * (mc + 1)],
                                    kt_[off:off + 64, 128 * nt:128 * (nt + 1)],
                                    qt[off:off + 64, 512 * mc:512 * (mc + 1)],
                                    start=True, stop=True,
                                    tile_position=(off, 0))
                            e = e_pool.tile([128, M], F32R, tag="E",
                                            name=f"E_{s}_{nt}_{hh}")
                            nc.scalar.activation(out=e, in_=s_ps, func=EXP,
                                                 scale=0.125)
                            E[s, hh, nt] = e
                        out.append(emit)
                return out

            def beta_chunks(s):
                chunks = []

                def mkstage(s=s):
                    stages[s] = stg_pool.tile([64, M], F32R, tag="stg",
                                              name=f"stg{s}")
                chunks.append(mkstage)
                for hh in range(2):
                    for mc in range(2):
                        unit = {}

                        def zblock(s=s, hh=hh, mc=mc, unit=unit):
                            mcs = slice(512 * mc, 512 * (mc + 1))
                            z_ps = z_pool.tile([32, 512], F32, tag="z",
                                               name=f"z_{s}_{hh}_{mc}")
                            for nt in range(NT):
                                nc.tensor.matmul(
                                    z_ps, i4_sb[:, 32 * nt:32 * (nt + 1)],
                                    E[s, hh, nt][:, mcs],
                                    start=(nt == 0), stop=(nt == 7))
                            rz32 = rz_pool.tile([32, 512], F32, tag="rz32",
                                                name=f"rz32_{s}_{hh}_{mc}")
                            nc.vector.reciprocal_approx_fast(out=rz32, in_=z_ps)
                            rz = rz_pool.tile([32, 512], F32R, tag="rz",
                                              name=f"rz_{s}_{hh}_{mc}")
                            nc.vector.tensor_copy(rz, rz32)
                            rzs = []
                            for r in range(2):
                                t = rzs_pool.tile([128, 512], F32R, tag="rzs",
                                                  name=f"rzs_{s}_{hh}_{mc}_{r}")
                                for a in range(4):
                                    nc.sync.dma_start(
                                        out=t[32 * a:32 * a + 4, :],
                                        in_=rz[16 * r + 4 * a:
                                               16 * r + 4 * a + 4, :])
                                rzs.append(t)
                            unit["rzs"] = rzs
                            unit["pv"] = pv_pool.tile(
                                [64, 512], F32, tag="pv",
                                name=f"pv_{s}_{hh}_{mc}")
                        chunks.append(zblock)

                        for nt in range(NT):
                            def step(s=s, hh=hh, mc=mc, nt=nt, unit=unit):
                                h = 2 * s + hh
                                mcs = slice(512 * mc, 512 * (mc + 1))
                                r, a = nt // 4, nt % 4
                                r_ps = r_pool.tile([128, 512], F32, tag="r",
                                                   name=f"rps_{s}_{hh}_{mc}_{nt}")
                                nc.tensor.matmul(
                                    r_ps, sel_sb[32 * a:32 * a + 4, :],
                                    unit["rzs"][r][32 * a:32 * a + 4, :],
                                    start=True, stop=True,
                                    tile_position=(32 * a, 0))
                                nc.vector.tensor_mul(
                                    out=E[s, hh, nt][:, mcs],
                                    in0=E[s, hh, nt][:, mcs], in1=r_ps)
                                nc.tensor.matmul(
                                    unit["pv"], v_sb[nt][:, 64 * h:64 * (h + 1)],
                                    E[s, hh, nt][:, mcs],
                                    start=(nt == 0), stop=(nt == 7))
                            chunks.append(step)

                        def copyout(s=s, hh=hh, mc=mc, unit=unit):
                            mcs = slice(512 * mc, 512 * (mc + 1))
                            if hh == 0:
                                nc.scalar.activation(
                                    out=outT_sb[s][0:64, mcs],
                                    in_=unit["pv"], func=COPY)
                            else:
                                nc.scalar.activation(
                                    out=stages[s][:, mcs],
                                    in_=unit["pv"], func=COPY)
                        chunks.append(copyout)

                def merge(s=s):
                    nc.sync.dma_start(out=outT_sb[s][64:128, :], in_=stages[s])
                chunks.append(merge)
                return chunks

            # software-pipeline: alpha(s) interleaved with beta(s-1)
            npair = 4
            for s in range(npair + 1):
                a = alpha_chunks(s) if s < npair and parts != "front" else []
                b = beta_chunks(s - 1) if s >= 1 and parts != "alpha" else []
                na, nb = len(a), len(b)
                if not a:
                    for f in b:
                        f()
                else:
                    ratio = nb / na if na else 0
                    bi = 0.0
                    for i, f in enumerate(a):
                        f()
                        target = (i + 1) * ratio
                        while bi < target and int(bi) < nb:
                            b[int(bi)]()
                            bi += 1
                    for j in range(int(bi), nb):
                        b[j]()

        # ================= PROJ =================
        if parts == "all":
          with ExitStack() as pj:
            pj_pool = pj.enter_context(tc.tile_pool(name="pj_ps", bufs=2, space="PSUM"))
            y_pool = pj.enter_context(tc.tile_pool(name="y_sb", bufs=3))
            for mt in range(NT):
                p = pj_pool.tile([128, DIM], F32, tag="pj")
                for kt in range(4):
                    nc.tensor.matmul(p,
                                     outT_sb[kt][:, 128 * mt:128 * (mt + 1)],
                                     wo_sb[kt], start=(kt == 0), stop=False)
                nc.tensor.matmul(p, ones_sb[:, 128 * mt:128 * (mt + 1)],
                                 bout_sb, start=False, stop=True)
                y_sb = y_pool.tile([128, DIM], F32, tag="y")
                nc.scalar.activation(out=y_sb, in_=p, func=COPY)
                nc.sync.dma_start(out=y[128 * mt:128 * (mt + 1), :], in_=y_sb)

    nc.compile()
    return nc


def _consts():
    ident = np.eye(128, dtype=np.float32)
    i4all = np.zeros((128, 256), np.float32)
    for nt in range(8):
        for p in range(128):
            i4all[p, 32 * nt + 4 * nt + p // 32] = 1.0
    sel = np.zeros((128, 128), np.float32)
    for p in range(128):
        if p % 32 < 4:
            for c in range(128):
                if c // 32 == p % 32:
                    sel[p, c] = 1.0
    ones_r = np.ones((1, M), np.float32)
    return ident, i4all, sel, ones_r


def kernel(x, w_qkv, w_out, b_out):
    from concourse import bass_utils
    if "nc" not in _CACHE:
        _CACHE["nc"] = _build()
    nc = _CACHE["nc"]
    ident, i4all, sel, ones_r = _consts()
    x = np.asarray(x, dtype=np.float32)
    in_maps = []
    for c in range(8):
        in_maps.append({
            "x": np.ascontiguousarray(x[c].reshape(M, DIM)),
            "w_qkv": np.asarray(w_qkv, np.float32),
            "w_out": np.asarray(w_out, np.float32),
            "b_out": np.asarray(b_out, np.float32).reshape(1, DIM),
            "ident": ident, "i4all": i4all, "sel": sel, "ones_r": ones_r,
        })
    res = bass_utils.run_bass_kernel_spmd(nc, in_maps, core_ids=list(range(8)))
    out = np.stack([res.results[c]["y"].reshape(H, W, DIM) for c in range(8)])
    return out

